# revision 25
# baseline (speedup 1.0000x reference)
"""AntiSymmetric GNN (2x AntiSymmetricConv + linear layers + log_softmax)
distributed Bass kernel for 8 TRN2 NeuronCores.

Strategy (v2):
  - Nodes sharded by destination across 8 cores (12500/core, padded 12544).
  - Edges partitioned by destination core; per core bucketed into
    (source-quarter, dest-window) segments, padded to the cross-core max
    (LMAX) with cycled real indices so the instruction stream is SPMD.
  - Per quarter the 98 window segments are CONCATENATED into one stream;
    dma_gather runs in large ~CALL_TILES*128-row calls (ring allows
    num_idxs/16+1 descs of 1024), cutting the ~1.2us/call Q7 fixed cost
    ~20x vs per-window gathers. Calls rotate the 4 SWDGE queues.
  - 128-row tiles of the stream may span window boundaries; the host emits
    a static op list (tile, window, colv column, start, stop); the one-hot
    scatter matrices S are built by one batched is_equal per call against
    a per-op colv table (rows outside the op's window get -1 -> S row 0).
  - Aggregation across the 4 quarter passes uses a bf16 agg carrier in
    SBUF: each window chain begins with an identity-matmul "fold" of the
    carrier into PSUM and ends with a scalar-engine drain (4 windows per
    PSUM bank) back to the carrier.  No vector-engine adds at all.
  - gcn norm factorizes: agg[c] = dinv[c] * (sum_e T[src_e] + T[c]),
    T = dinv*xw; phase A writes the self-loop term T[c] into the carrier.
  - Both layers' tables are 256B bf16 rows ([*,HID] and [*,128] zero
    padded) so one gather path serves both layers.
  - Tables are AllGathered in 4 window-aligned chunks (<=25600 rows so
    gather indices fit int16); layer-2 table chunks are produced by the
    phase-D callbacks embedded in layer-1's last pass, and the final
    conv+log_softmax runs in callbacks of layer-2's last pass.
"""

import numpy as np
import ml_dtypes

N = 100_000
F_IN = 256
HID = 128
C = 32
EPS = 0.1
GAMMA = 0.1

NCORES = 8
SHARD = 12_500
PADN = 12_544            # 98 * 128
W = 98                   # windows per core
QW = [25, 25, 24, 24]    # windows per quarter chunk (table chunks)
QROWS = [3200, 3200, 3072, 3072]
QSTART = [0, 3200, 6400, 9472]
QWSTART = [0, 25, 50, 74]
QWEND = [24, 49, 73, 97]  # last window of each t2-quarter chunk

CALL_TILES = 24          # tiles (128 rows) per dma_gather call
GRP = 4                  # windows per PSUM drain group

_CACHE = {}


def _host_prep(x, lin1_w, lin1_b, lin2_w, lin2_b, W1, phi1_w, b1, W2, phi2_w, b2,
               edge_index):
    bf16 = ml_dtypes.bfloat16
    row = edge_index[0].astype(np.int64)
    col = edge_index[1].astype(np.int64)

    # degrees INCLUDE self loops (reference appends them)
    deg = (np.bincount(col, minlength=N) + 1).astype(np.float32)
    dinv = 1.0 / np.sqrt(deg)

    # source -> (quarter chunk, int16 index into chunk table)
    ks = row // SHARD
    i_s = row % SHARD
    wloc = i_s // 128
    q_s = np.where(wloc < 25, 0, np.where(wloc < 50, 1, np.where(wloc < 74, 2, 3)))
    pos = i_s - np.asarray(QSTART)[q_s]
    idx16_all = ks * np.asarray(QROWS)[q_s] + pos

    k_dst = col // SHARD

    cores = []
    L = np.zeros((NCORES, 4 * W), np.int64)
    for k in range(NCORES):
        m = k_dst == k
        r_idx = idx16_all[m]
        c_loc = col[m] - k * SHARD
        key = q_s[m] * W + c_loc // 128
        # within each segment, order by table row so gather descriptors
        # hit ascending HBM addresses (DRAM locality)
        order = np.lexsort((r_idx, key))
        cores.append((key[order], r_idx[order],
                      (c_loc % 128)[order].astype(np.float32)))
        L[k] = np.bincount(cores[k][0], minlength=4 * W)

    # every (quarter, window) keeps >=1 row so every window has an op in
    # every pass (keeps the fold/drain grouping uniform)
    LMAX = np.maximum(L.max(axis=0), 1)

    # ---- shared (LMAX-derived) stream layout + op lists per quarter ----
    seg_start = np.zeros((4, W), np.int64)
    rows_q = []          # padded rows per quarter
    ntiles_q = []
    ops_q = []           # per q: (op_tile, op_w, op_start, op_stop) arrays
    calls_q = []         # per q: list of (t0, nt)
    row_window = []      # per q: [rows] window id of each stream row
    for q in range(4):
        off = 0
        rw = []
        for w in range(W):
            seg_start[q, w] = off
            lm = int(LMAX[q * W + w])
            rw.append(np.full(lm, w, np.int32))
            off += lm
        rows_pad = -(-off // 128) * 128
        rw.append(np.full(rows_pad - off, -1, np.int32))
        rwin = np.concatenate(rw)
        nt = rows_pad // 128
        ot, ow, ost, osp = [], [], [], []
        for w in range(W):
            a = int(seg_start[q, w])
            b = a + int(LMAX[q * W + w])
            t0, t1 = a // 128, (b - 1) // 128
            for t in range(t0, t1 + 1):
                ot.append(t)
                ow.append(w)
                ost.append(t == t0)
                osp.append(t == t1)
        rows_q.append(rows_pad)
        ntiles_q.append(nt)
        ops_q.append((np.asarray(ot), np.asarray(ow),
                      np.asarray(ost), np.asarray(osp)))
        calls_q.append([(t0, min(CALL_TILES, nt - t0))
                        for t0 in range(0, nt, CALL_TILES)])
        row_window.append(rwin)

    def wrap_idx(arr):
        a16 = arr.reshape(-1, 16).T
        return np.ascontiguousarray(np.tile(a16, (8, 1)))

    # ---- per-core data ----
    in_maps = []
    for k in range(NCORES):
        key_s, idx_s, cl_s = cores[k]
        starts_src = np.zeros(4 * W + 1, np.int64)
        np.cumsum(L[k], out=starts_src[1:])
        im = {}
        for q in range(4):
            rows_pad = rows_q[q]
            idx_arr = np.zeros(rows_pad, np.int16)
            rcolv = np.full(rows_pad, -1.0, np.float32)
            for w in range(W):
                s = q * W + w
                a = int(seg_start[q, w])
                lk = int(L[k][s])
                lm = int(LMAX[s])
                if lk > 0:
                    src0 = int(starts_src[s])
                    seg_idx = idx_s[src0:src0 + lk].astype(np.int16)
                    idx_arr[a:a + lk] = seg_idx
                    rcolv[a:a + lk] = cl_s[src0:src0 + lk]
                    if lm > lk:
                        idx_arr[a + lk:a + lm] = np.resize(seg_idx, lm - lk)
            ops_t, ops_w, _, _ = ops_q[q]
            tile_rows = ops_t[:, None] * 128 + np.arange(128)[None, :]
            cm = np.where(row_window[q][tile_rows] == ops_w[:, None],
                          rcolv[tile_rows], -1.0)           # [nops, 128]
            # duplicated x2 so the kernel's one-hot compare has inner step 1
            # on every operand (DVE 2x mode requires it)
            cm2 = np.repeat(cm.T.astype(bf16)[:, :, None], 2, axis=2)
            im[f"idx{q}"] = wrap_idx(idx_arr)
            im[f"colv{q}"] = np.ascontiguousarray(cm2.reshape(128, -1))

        xs = np.zeros((PADN, F_IN), np.float32)
        xs[:SHARD] = x[k * SHARD:(k + 1) * SHARD]
        dvk = np.zeros(PADN, np.float32)
        dvk[:SHARD] = dinv[k * SHARD:(k + 1) * SHARD]
        im.update({
            "xT": np.ascontiguousarray(xs.T).astype(bf16),
            "dinv_w": np.ascontiguousarray(dvk.reshape(W, 128).T),
            "lin1T": np.ascontiguousarray(lin1_w.T).astype(bf16),
            "phi1T": np.ascontiguousarray(phi1_w.T).astype(bf16),
            "aw1T": np.ascontiguousarray(
                (W1 - W1.T - GAMMA * np.eye(HID, dtype=np.float32)).T).astype(bf16),
            "lin2T": np.ascontiguousarray(lin2_w.T).astype(bf16),
            "phi2T": np.ascontiguousarray(phi2_w.T).astype(bf16),
            "aw2T": np.ascontiguousarray(
                (W2 - W2.T - GAMMA * np.eye(C, dtype=np.float32)).T).astype(bf16),
        })
        in_maps.append(im)

    biases = {
        "blin1": np.broadcast_to(lin1_b, (128, HID)).astype(np.float32).copy(),
        "bconv1": np.broadcast_to(b1, (128, HID)).astype(np.float32).copy(),
        "blin2": np.broadcast_to(lin2_b, (128, C)).astype(np.float32).copy(),
        "bconv2": np.broadcast_to(b2, (128, C)).astype(np.float32).copy(),
    }
    use_bias = {name: bool(np.any(arr)) for name, arr in biases.items()}
    for name, used in use_bias.items():
        if used:
            for im in in_maps:
                im[name] = biases[name]

    meta = {
        "LMAX": LMAX,
        "rows_q": rows_q, "ntiles_q": ntiles_q,
        "ops_q": ops_q, "calls_q": calls_q,
        "use_bias": use_bias,
    }
    return in_maps, meta


def _build_graph(meta):
    import concourse.bass as bass
    import concourse.mybir as mybir
    import concourse.tile as tile
    from concourse import bacc
    from concourse.masks import make_identity
    from contextlib import ExitStack

    dt = mybir.dt
    Alu = mybir.AluOpType
    Act = mybir.ActivationFunctionType
    rows_q = meta["rows_q"]
    ops_q = meta["ops_q"]
    calls_q = meta["calls_q"]
    use_bias = meta["use_bias"]

    nc = bacc.Bacc("TRN2", target_bir_lowering=False, num_swdge_queues=4,
                   dynamic_dma_scratch_size=32768)

    xT = nc.declare_dram_parameter("xT", [F_IN, PADN], dt.bfloat16, isOutput=False)
    dinv_w = nc.declare_dram_parameter("dinv_w", [128, W], dt.float32, isOutput=False)
    lin1T = nc.declare_dram_parameter("lin1T", [F_IN, HID], dt.bfloat16, isOutput=False)
    phi1T = nc.declare_dram_parameter("phi1T", [HID, HID], dt.bfloat16, isOutput=False)
    aw1T = nc.declare_dram_parameter("aw1T", [HID, HID], dt.bfloat16, isOutput=False)
    lin2T = nc.declare_dram_parameter("lin2T", [HID, C], dt.bfloat16, isOutput=False)
    phi2T = nc.declare_dram_parameter("phi2T", [C, C], dt.bfloat16, isOutput=False)
    aw2T = nc.declare_dram_parameter("aw2T", [C, C], dt.bfloat16, isOutput=False)
    idx_p, colv_p = [], []
    for q in range(4):
        nops = len(ops_q[q][0])
        idx_p.append(nc.declare_dram_parameter(
            f"idx{q}", [128, rows_q[q] // 16], dt.int16, isOutput=False))
        colv_p.append(nc.declare_dram_parameter(
            f"colv{q}", [128, 2 * nops], dt.bfloat16, isOutput=False))
    bias_p = {}
    for name, shape in [("blin1", [128, HID]), ("bconv1", [128, HID]),
                        ("blin2", [128, C]), ("bconv2", [128, C])]:
        if use_bias[name]:
            bias_p[name] = nc.declare_dram_parameter(name, shape, dt.float32,
                                                     isOutput=False)
    out_p = nc.declare_dram_parameter("out", [PADN, C], dt.float32, isOutput=True)

    t1q_in = [nc.dram_tensor(f"t1in{q}", [QROWS[q], HID], dt.bfloat16)
              for q in range(4)]
    t1q_tab = [nc.dram_tensor(f"t1tab{q}", [NCORES * QROWS[q], HID], dt.bfloat16,
                              addr_space="Shared") for q in range(4)]
    t2q_in = [nc.dram_tensor(f"t2in{q}", [QROWS[q], 128], dt.bfloat16)
              for q in range(4)]
    t2q_tab = [nc.dram_tensor(f"t2tab{q}", [NCORES * QROWS[q], 128], dt.bfloat16,
                              addr_space="Shared") for q in range(4)]

    rg = [list(range(NCORES))]

    with tile.TileContext(nc) as tc, ExitStack() as top:
        const = top.enter_context(tc.tile_pool(name="const", bufs=1))
        big = top.enter_context(tc.tile_pool(name="big", bufs=1))
        tmp_pool = top.enter_context(tc.tile_pool(name="tmp", bufs=3))

        lin1T_sb = const.tile([128, 2, HID], dt.bfloat16)
        nc.sync.dma_start(lin1T_sb[:], lin1T[:].rearrange("(t p) j -> p t j", p=128))
        phi1T_sb = const.tile([128, HID], dt.bfloat16)
        nc.sync.dma_start(phi1T_sb[:], phi1T[:])
        aw1T_sb = const.tile([128, HID], dt.bfloat16)
        nc.sync.dma_start(aw1T_sb[:], aw1T[:])
        lin2T_sb = const.tile([128, C], dt.bfloat16)
        nc.sync.dma_start(lin2T_sb[:], lin2T[:])
        phi2T_sb = const.tile([128, C], dt.bfloat16)
        aw2T_sb = const.tile([128, C], dt.bfloat16)
        for r in range(4):
            nc.sync.dma_start(phi2T_sb[r * C:(r + 1) * C, :], phi2T[:])
            nc.sync.dma_start(aw2T_sb[r * C:(r + 1) * C, :], aw2T[:])
        dinv_sb = const.tile([128, W], dt.float32)
        nc.sync.dma_start(dinv_sb[:], dinv_w[:])
        bias_sb = {}
        for name, p in bias_p.items():
            t = const.tile(list(p.shape), dt.float32)
            nc.sync.dma_start(t[:], p[:])
            bias_sb[name] = t

        iota_i = const.tile([128, 128], dt.int32)
        nc.gpsimd.iota(iota_i[:], pattern=[[1, 128]], base=0, channel_multiplier=0)
        iota_bf = const.tile([128, 128], dt.bfloat16)
        nc.vector.tensor_copy(iota_bf[:], iota_i[:])
        ident = const.tile([128, 128], dt.bfloat16)
        make_identity(nc, ident[:])

        # persistent state
        h1 = big.tile([128, W, HID], dt.bfloat16)
        h1T_all = big.tile([128, W, HID], dt.bfloat16, tag="h1T_all")
        agg = big.tile([128, W, HID], dt.bfloat16, tag="agg")        # L1 carrier
        h2 = big.tile([128, W, C], dt.bfloat16, tag="h2")
        agg2 = big.tile([128, W, C], dt.bfloat16, tag="agg2")        # L2 carrier
        h2T_all = big.tile([128, (W + 1) // 2, 128], dt.bfloat16, tag="h2T_all")

        SOPS = max(o1 - o0 for q in range(4)
                   for o0, o1 in _call_op_ranges(ops_q[q], calls_q[q]))

        with tc.tile_pool(name="gp", bufs=7) as gp, \
             tc.tile_pool(name="sp", bufs=2) as sp, \
             tc.tile_pool(name="ip", bufs=4) as ip, \
             tc.tile_pool(name="cp", bufs=2) as cp, \
             tc.tile_pool(name="psC", bufs=4, space="PSUM") as psC, \
             ExitStack() as stA, ExitStack() as stD:

            xqp = stA.enter_context(tc.tile_pool(name="xq", bufs=1))
            psA = stA.enter_context(tc.tile_pool(name="psA", bufs=1,
                                                 space="PSUM"))
            psAT = stA.enter_context(tc.tile_pool(name="psAT", bufs=1,
                                                  space="PSUM"))

            # ===== Phase A: h1, h1T, T1 (self term into agg) + AllGather =====
            for q in range(4):
                c0 = QWSTART[q] * 128
                cw = QW[q] * 128
                xq_sb = xqp.tile([128, 2, 25 * 128], dt.bfloat16, tag="xq")
                nc.sync.dma_start(
                    xq_sb[:, :, 0:cw],
                    xT[:, c0:c0 + cw].rearrange("(t p) c -> p t c", p=128))
                for wi in range(QW[q]):
                    w = QWSTART[q] + wi
                    ph = psA.tile([128, HID], dt.float32, tag="ph")
                    nc.tensor.matmul(ph[:], xq_sb[:, 0, wi * 128:(wi + 1) * 128],
                                     lin1T_sb[:, 0, :], start=True, stop=False)
                    nc.tensor.matmul(ph[:], xq_sb[:, 1, wi * 128:(wi + 1) * 128],
                                     lin1T_sb[:, 1, :], start=False, stop=True)
                    if "blin1" in bias_sb:
                        t = tmp_pool.tile([128, HID], dt.float32, tag="tA")
                        nc.vector.tensor_tensor(t[:], ph[:], bias_sb["blin1"][:],
                                                op=Alu.add)
                        nc.scalar.activation(h1[:, w, :], t[:], Act.Relu)
                    else:
                        nc.scalar.activation(h1[:, w, :], ph[:], Act.Relu)
                    pt = psAT.tile([128, 128], dt.bfloat16, tag="pt")
                    nc.tensor.transpose(pt[:], h1[:, w, :], ident[:])
                    nc.scalar.copy(h1T_all[:, w, :], pt[:])
                    pT = psA.tile([128, HID], dt.float32, tag="pT1")
                    nc.tensor.matmul(pT[:], h1T_all[:, w, :], phi1T_sb[:],
                                     start=True, stop=True)
                    nc.scalar.activation(agg[:, w, :], pT[:], Act.Copy,
                                         scale=dinv_sb[:, w:w + 1])
                nc.sync.dma_start(
                    t1q_in[q][:].rearrange("(w p) f -> p w f", p=128),
                    agg[:, QWSTART[q]:QWSTART[q] + QW[q], :])
                nc.gpsimd.collective_compute(
                    "AllGather", Alu.bypass, replica_groups=rg,
                    ins=[t1q_in[q][:].opt()], outs=[t1q_tab[q][:].opt()])

            stA.close()

            # ===== aggregation pass over one quarter's stream =====
            qctr = [0]

            def aggregate(table, fw, agg_t, q, post_cb=None):
                op_t, op_w, op_st, op_sp = ops_q[q]
                ranges = _call_op_ranges(ops_q[q], calls_q[q])
                colv_sb = cp.tile([128, 2 * len(op_t)], dt.bfloat16, tag="colv")
                nc.sync.dma_start(colv_sb[:], colv_p[q][:])
                pseg = {}
                for ci, (t0, nt) in enumerate(calls_q[q]):
                    rows = nt * 128
                    idx_sb = ip.tile([128, CALL_TILES * 8], dt.int16, tag="idx")
                    nc.sync.dma_start(idx_sb[:, 0:nt * 8],
                                      idx_p[q][:, t0 * 8:t0 * 8 + nt * 8])
                    g = gp.tile([128, CALL_TILES, 128], dt.bfloat16, tag="g")
                    nc.gpsimd.dma_gather(
                        g[:, 0:nt, :], table[q][:], idx_sb[:, 0:nt * 8],
                        rows, rows, 128, queue_num=qctr[0] % 4,
                        single_packet=False)
                    qctr[0] += 1
                    o0, o1 = ranges[ci]
                    no = o1 - o0
                    S = sp.tile([128, SOPS, 128], dt.bfloat16, tag="S")
                    # every operand has inner step 1 (DVE 2x_1P mode):
                    # S[p,o,h,l] = (iota[h,l] == colv2[p,o,l])
                    nc.vector.tensor_tensor(
                        S[:, 0:no, :].rearrange("p o (h l) -> p o h l", l=2),
                        iota_bf[:].rearrange("p (h l) -> p h l", l=2)
                            .unsqueeze(1).broadcast_to([128, no, 64, 2]),
                        colv_sb[:, 2 * o0:2 * o1]
                            .rearrange("p (o l) -> p o l", l=2)
                            .unsqueeze(2).broadcast_to([128, no, 64, 2]),
                        op=Alu.is_equal)
                    for o in range(o0, o1):
                        w = int(op_w[o])
                        grp = w // GRP
                        slot = w % GRP
                        if op_st[o]:
                            if grp not in pseg:
                                pseg[grp] = psC.tile([128, GRP, fw], dt.float32,
                                                     name="pseg", tag="pseg")
                            nc.tensor.matmul(pseg[grp][:, slot, :], ident[:],
                                             agg_t[:, w, 0:fw],
                                             start=True, stop=False)
                        nc.tensor.matmul(pseg[grp][:, slot, :],
                                         S[:, o - o0, :],
                                         g[:, int(op_t[o]) - t0, 0:fw],
                                         start=False, stop=bool(op_sp[o]))
                        if op_sp[o] and (slot == GRP - 1 or w == W - 1):
                            w0 = grp * GRP
                            gn = w - w0 + 1
                            nc.scalar.copy(agg_t[:, w0:w0 + gn, :],
                                           pseg[grp][:, 0:gn, :])
                            del pseg[grp]
                            if post_cb is not None:
                                post_cb(w0, gn)

            # ===== Phase C: layer-1 aggregation, passes 0..2 =====
            for q in range(3):
                aggregate(t1q_tab, HID, agg, q)

            # ===== Phase D machinery (runs in pass-3 callbacks) =====
            t2qbp = stD.enter_context(tc.tile_pool(name="t2qb", bufs=2))
            dstg = stD.enter_context(tc.tile_pool(name="dstg", bufs=2))
            psD = stD.enter_context(tc.tile_pool(name="psD", bufs=1,
                                                 space="PSUM"))
            psDt = stD.enter_context(tc.tile_pool(name="psDt", bufs=1,
                                                  space="PSUM"))
            # zero both rotating t2qb bufs once; cols C:128 stay zero
            for _ in range(2):
                z = t2qbp.tile([128, 25, 128], dt.bfloat16, tag="t2qb")
                nc.vector.memset(z[:], 0.0)

            dstate = {}

            def d_window(w):
                Q = 0 if w < 25 else (1 if w < 50 else (2 if w < 74 else 3))
                if dstate.get("Q") != Q:
                    dstate["Q"] = Q
                    dstate["t2qb"] = t2qbp.tile([128, 25, 128], dt.bfloat16,
                                                name="t2qb", tag="t2qb")
                i = w - QWSTART[Q]
                # s1: h1' = h1 + eps*tanh(aw@.. + dinv*agg)
                paw = psD.tile([128, HID], dt.float32, tag="paw")
                nc.tensor.matmul(paw[:], h1T_all[:, w, :], aw1T_sb[:],
                                 start=True, stop=True)
                pre = tmp_pool.tile([128, HID], dt.float32, tag="pre")
                nc.vector.scalar_tensor_tensor(
                    pre[:], agg[:, w, :], dinv_sb[:, w:w + 1], paw[:],
                    op0=Alu.mult, op1=Alu.add)
                if "bconv1" in bias_sb:
                    nc.vector.tensor_tensor(
                        pre[:], pre[:], bias_sb["bconv1"][:], op=Alu.add)
                th = tmp_pool.tile([128, HID], dt.float32, tag="th")
                nc.scalar.activation(th[:], pre[:], Act.Tanh)
                h1p = dstg.tile([128, HID], dt.bfloat16, tag="h1p")
                nc.vector.scalar_tensor_tensor(
                    h1p[:], th[:], EPS, h1[:, w, :],
                    op0=Alu.mult, op1=Alu.add)
                # s2: h2 = h1' @ lin2
                pt2 = psDt.tile([128, 128], dt.bfloat16, tag="ptD")
                nc.tensor.transpose(pt2[:], h1p[:], ident[:])
                h1pT = tmp_pool.tile([128, 128], dt.bfloat16, tag="h1pT")
                nc.scalar.copy(h1pT[:], pt2[:])
                ph2 = psD.tile([128, C], dt.float32, tag="pD32")
                nc.tensor.matmul(ph2[:], h1pT[:], lin2T_sb[:],
                                 start=True, stop=True)
                if "blin2" in bias_sb:
                    nc.vector.tensor_tensor(
                        h2[:, w, :], ph2[:], bias_sb["blin2"][:], op=Alu.add)
                else:
                    nc.scalar.copy(h2[:, w, :], ph2[:])
                # s3: h2T, T2 = dinv * (h2 @ phi2^T); agg2 self term
                p0 = (w % 2) * 64
                pt3 = psDt.tile([C, 128], dt.bfloat16, tag="pt3")
                nc.tensor.transpose(pt3[:], h2[:, w, :], ident[:])
                nc.scalar.copy(h2T_all[p0:p0 + C, w // 2, :], pt3[:])
                pT2 = psD.tile([128, C], dt.float32, tag="pD32")
                nc.tensor.matmul(pT2[:], h2T_all[p0:p0 + C, w // 2, :],
                                 phi2T_sb[p0:p0 + C, :],
                                 start=True, stop=True)
                nc.scalar.activation(dstate["t2qb"][:, i, 0:C], pT2[:],
                                     Act.Copy, scale=dinv_sb[:, w:w + 1])
                nc.scalar.activation(agg2[:, w, :], pT2[:],
                                     Act.Copy, scale=dinv_sb[:, w:w + 1])
                if w == QWEND[Q]:
                    nc.sync.dma_start(
                        t2q_in[Q][:].rearrange("(w p) f -> p w f", p=128),
                        dstate["t2qb"][:, 0:QW[Q], :])
                    nc.gpsimd.collective_compute(
                        "AllGather", Alu.bypass, replica_groups=rg,
                        ins=[t2q_in[Q][:].opt()], outs=[t2q_tab[Q][:].opt()])

            def d_cb(w0, gn):
                for i in range(gn):
                    d_window(w0 + i)

            # layer-1 pass 3 with phase-D callbacks
            aggregate(t1q_tab, HID, agg, 3, post_cb=d_cb)

            # ===== Phase G (runs in layer-2 pass-3 callbacks) =====
            def g_group(w0, gw):
                a1 = tmp_pool.tile([128, GRP, C], dt.float32, tag="a1g")
                nc.vector.tensor_tensor(
                    a1[:, 0:gw, :], agg2[:, w0:w0 + gw, :],
                    dinv_sb[:, w0:w0 + gw].unsqueeze(2)
                        .broadcast_to([128, gw, C]),
                    op=Alu.mult)
                pre = tmp_pool.tile([128, GRP, C], dt.float32, tag="preg")
                for wi in range(gw):
                    w = w0 + wi
                    p0 = (w % 2) * 64
                    pawt = psD.tile([128, C], dt.float32, tag="pD32")
                    nc.tensor.matmul(pawt[:],
                                     h2T_all[p0:p0 + C, w // 2, :],
                                     aw2T_sb[p0:p0 + C, :],
                                     start=True, stop=True)
                    nc.vector.tensor_tensor(
                        pre[:, wi, :], a1[:, wi, :], pawt[:], op=Alu.add)
                if "bconv2" in bias_sb:
                    nc.vector.tensor_tensor(
                        pre[:, 0:gw, :], pre[:, 0:gw, :],
                        bias_sb["bconv2"][:].unsqueeze(1)
                            .broadcast_to([128, gw, C]),
                        op=Alu.add)
                th = tmp_pool.tile([128, GRP, C], dt.float32, tag="thg")
                nc.scalar.activation(th[:, 0:gw, :], pre[:, 0:gw, :], Act.Tanh)
                h2p = tmp_pool.tile([128, GRP, C], dt.float32, tag="h2pg")
                nc.vector.scalar_tensor_tensor(
                    h2p[:, 0:gw, :], th[:, 0:gw, :], EPS,
                    h2[:, w0:w0 + gw, :], op0=Alu.mult, op1=Alu.add)
                negmax = tmp_pool.tile([128, GRP, 1], dt.float32, tag="nmg")
                nc.vector.tensor_reduce(negmax[:, 0:gw, :], h2p[:, 0:gw, :],
                                        axis=mybir.AxisListType.X,
                                        op=Alu.max, negate=True)
                sub = tmp_pool.tile([128, GRP, C], dt.float32, tag="subg")
                nc.vector.tensor_tensor(
                    sub[:, 0:gw, :], h2p[:, 0:gw, :],
                    negmax[:, 0:gw, :].broadcast_to([128, gw, C]),
                    op=Alu.add)
                e = tmp_pool.tile([128, GRP, C], dt.float32, tag="eg")
                nc.scalar.activation(e[:, 0:gw, :], sub[:, 0:gw, :], Act.Exp)
                ssum = tmp_pool.tile([128, GRP, 1], dt.float32, tag="ssg")
                nc.vector.tensor_reduce(ssum[:, 0:gw, :], e[:, 0:gw, :],
                                        axis=mybir.AxisListType.X,
                                        op=Alu.add)
                lse = tmp_pool.tile([128, GRP, 1], dt.float32, tag="lseg")
                nc.scalar.activation(lse[:, 0:gw, :], ssum[:, 0:gw, :], Act.Ln)
                fin = tmp_pool.tile([128, GRP, C], dt.float32, tag="fing")
                nc.vector.tensor_tensor(
                    fin[:, 0:gw, :], sub[:, 0:gw, :],
                    lse[:, 0:gw, :].broadcast_to([128, gw, C]),
                    op=Alu.subtract)
                nc.sync.dma_start(
                    out_p[w0 * 128:(w0 + gw) * 128, :]
                        .rearrange("(w p) c -> p w c", p=128),
                    fin[:, 0:gw, :])

            # ===== Phase F: layer-2 aggregation =====
            for q in range(4):
                aggregate(t2q_tab, C, agg2, q,
                          post_cb=g_group if q == 3 else None)

    nc.compile()
    return nc


def _call_op_ranges(ops, calls):
    """[o0, o1) op index range per gather call (ops sorted by tile)."""
    op_t = ops[0]
    ranges = []
    for t0, nt in calls:
        o0 = int(np.searchsorted(op_t, t0, side="left"))
        o1 = int(np.searchsorted(op_t, t0 + nt - 1, side="right"))
        ranges.append((o0, o1))
    return ranges


def kernel(**inputs):
    from concourse.bass_utils import run_bass_kernel_spmd

    inp = {k: np.asarray(v) for k, v in inputs.items()}
    in_maps, meta = _host_prep(**inp)

    key = ("graph", tuple(meta["LMAX"].tolist()),
           tuple(sorted(meta["use_bias"].items())))
    if key not in _CACHE:
        _CACHE[key] = _build_graph(meta)
    nc = _CACHE[key]

    import os
    trace = bool(int(os.environ.get("KERNEL_TRACE", "0")))
    res = run_bass_kernel_spmd(nc, in_maps, list(range(NCORES)), trace=trace,
                               tmpdir=os.environ.get("KERNEL_TRACE_DIR"))
    global LAST_EXEC_NS
    LAST_EXEC_NS = res.exec_time_ns

    out = np.concatenate([res.results[k]["out"][:SHARD] for k in range(NCORES)], 0)
    return out.astype(np.float32)


LAST_EXEC_NS = None


# revision 29
# speedup vs baseline: 1.1566x; 1.1566x over previous
"""AntiSymmetric GNN (2x AntiSymmetricConv + linear layers + log_softmax)
distributed Bass kernel for 8 TRN2 NeuronCores.

Strategy (v2):
  - Nodes sharded by destination across 8 cores (12500/core, padded 12544).
  - Edges partitioned by destination core; per core bucketed into
    (source-quarter, dest-window) segments, padded to the cross-core max
    (LMAX) with cycled real indices so the instruction stream is SPMD.
  - Per quarter the 98 window segments are CONCATENATED into one stream;
    dma_gather runs in large ~CALL_TILES*128-row calls (ring allows
    num_idxs/16+1 descs of 1024), cutting the ~1.2us/call Q7 fixed cost
    ~20x vs per-window gathers. Calls rotate the 4 SWDGE queues.
  - 128-row tiles of the stream may span window boundaries; the host emits
    a static op list (tile, window, colv column, start, stop); the one-hot
    scatter matrices S are built by one batched is_equal per call against
    a per-op colv table (rows outside the op's window get -1 -> S row 0).
  - Aggregation across the 4 quarter passes uses a bf16 agg carrier in
    SBUF: each window chain begins with an identity-matmul "fold" of the
    carrier into PSUM and ends with a scalar-engine drain (4 windows per
    PSUM bank) back to the carrier.  No vector-engine adds at all.
  - gcn norm factorizes: agg[c] = dinv[c] * (sum_e T[src_e] + T[c]),
    T = dinv*xw; phase A writes the self-loop term T[c] into the carrier.
  - Both layers' tables are 256B bf16 rows ([*,HID] and [*,128] zero
    padded) so one gather path serves both layers.
  - Tables are AllGathered in 4 window-aligned chunks (<=25600 rows so
    gather indices fit int16); layer-2 table chunks are produced by the
    phase-D callbacks embedded in layer-1's last pass, and the final
    conv+log_softmax runs in callbacks of layer-2's last pass.
"""

import numpy as np
import ml_dtypes

N = 100_000
F_IN = 256
HID = 128
C = 32
EPS = 0.1
GAMMA = 0.1

NCORES = 8
SHARD = 12_500
PADN = 12_544            # 98 * 128
W = 98                   # windows per core
QW = [25, 25, 24, 24]    # windows per quarter chunk (table chunks)
QROWS = [3200, 3200, 3072, 3072]
QSTART = [0, 3200, 6400, 9472]
QWSTART = [0, 25, 50, 74]
QWEND = [24, 49, 73, 97]  # last window of each t2-quarter chunk

CALL_TILES = 24          # tiles (128 rows) per dma_gather call
GRP = 4                  # windows per PSUM drain group

_CACHE = {}


def _host_prep(x, lin1_w, lin1_b, lin2_w, lin2_b, W1, phi1_w, b1, W2, phi2_w, b2,
               edge_index):
    bf16 = ml_dtypes.bfloat16
    row = edge_index[0].astype(np.int64)
    col = edge_index[1].astype(np.int64)

    # degrees INCLUDE self loops (reference appends them)
    deg = (np.bincount(col, minlength=N) + 1).astype(np.float32)
    dinv = 1.0 / np.sqrt(deg)

    # source -> (quarter chunk, int16 index into chunk table)
    ks = row // SHARD
    i_s = row % SHARD
    wloc = i_s // 128
    q_s = np.where(wloc < 25, 0, np.where(wloc < 50, 1, np.where(wloc < 74, 2, 3)))
    pos = i_s - np.asarray(QSTART)[q_s]
    idx16_all = ks * np.asarray(QROWS)[q_s] + pos

    k_dst = col // SHARD

    cores = []
    L = np.zeros((NCORES, 4 * W), np.int64)
    for k in range(NCORES):
        m = k_dst == k
        r_idx = idx16_all[m]
        c_loc = col[m] - k * SHARD
        key = q_s[m] * W + c_loc // 128
        # within each segment, order by table row so gather descriptors
        # hit ascending HBM addresses (DRAM locality)
        order = np.lexsort((r_idx, key))
        cores.append((key[order], r_idx[order],
                      (c_loc % 128)[order].astype(np.float32)))
        L[k] = np.bincount(cores[k][0], minlength=4 * W)

    # every (quarter, window) keeps >=1 row so every window has an op in
    # every pass (keeps the fold/drain grouping uniform)
    LMAX = np.maximum(L.max(axis=0), 1)

    # ---- shared (LMAX-derived) stream layout + op lists per quarter ----
    seg_start = np.zeros((4, W), np.int64)
    rows_q = []          # padded rows per quarter
    ntiles_q = []
    ops_q = []           # per q: (op_tile, op_w, op_start, op_stop) arrays
    calls_q = []         # per q: list of (t0, nt)
    row_window = []      # per q: [rows] window id of each stream row
    for q in range(4):
        off = 0
        rw = []
        for w in range(W):
            seg_start[q, w] = off
            lm = int(LMAX[q * W + w])
            rw.append(np.full(lm, w, np.int32))
            off += lm
        rows_pad = -(-off // 128) * 128
        rw.append(np.full(rows_pad - off, -1, np.int32))
        rwin = np.concatenate(rw)
        nt = rows_pad // 128
        ot, ow, ost, osp = [], [], [], []
        for w in range(W):
            a = int(seg_start[q, w])
            b = a + int(LMAX[q * W + w])
            t0, t1 = a // 128, (b - 1) // 128
            for t in range(t0, t1 + 1):
                ot.append(t)
                ow.append(w)
                ost.append(t == t0)
                osp.append(t == t1)
        rows_q.append(rows_pad)
        ntiles_q.append(nt)
        ops_q.append((np.asarray(ot), np.asarray(ow),
                      np.asarray(ost), np.asarray(osp)))
        calls_q.append([(t0, min(CALL_TILES, nt - t0))
                        for t0 in range(0, nt, CALL_TILES)])
        row_window.append(rwin)

    def wrap_idx(arr):
        a16 = arr.reshape(-1, 16).T
        return np.ascontiguousarray(np.tile(a16, (8, 1)))

    # ---- per-core data ----
    in_maps = []
    for k in range(NCORES):
        key_s, idx_s, cl_s = cores[k]
        starts_src = np.zeros(4 * W + 1, np.int64)
        np.cumsum(L[k], out=starts_src[1:])
        im = {}
        for q in range(4):
            rows_pad = rows_q[q]
            idx_arr = np.zeros(rows_pad, np.int16)
            rcolv = np.full(rows_pad, -1.0, np.float32)
            for w in range(W):
                s = q * W + w
                a = int(seg_start[q, w])
                lk = int(L[k][s])
                lm = int(LMAX[s])
                if lk > 0:
                    src0 = int(starts_src[s])
                    seg_idx = idx_s[src0:src0 + lk].astype(np.int16)
                    idx_arr[a:a + lk] = seg_idx
                    rcolv[a:a + lk] = cl_s[src0:src0 + lk]
                    if lm > lk:
                        idx_arr[a + lk:a + lm] = np.resize(seg_idx, lm - lk)
            ops_t, ops_w, _, _ = ops_q[q]
            tile_rows = ops_t[:, None] * 128 + np.arange(128)[None, :]
            cm = np.where(row_window[q][tile_rows] == ops_w[:, None],
                          rcolv[tile_rows], -1.0)           # [nops, 128]
            # duplicated x2 so the kernel's one-hot compare has inner step 1
            # on every operand (DVE 2x mode requires it)
            cm2 = np.repeat(cm.T.astype(bf16)[:, :, None], 2, axis=2)
            im[f"idx{q}"] = wrap_idx(idx_arr)
            im[f"colv{q}"] = np.ascontiguousarray(cm2.reshape(128, -1))

        xs = np.zeros((PADN, F_IN), np.float32)
        xs[:SHARD] = x[k * SHARD:(k + 1) * SHARD]
        dvk = np.zeros(PADN, np.float32)
        dvk[:SHARD] = dinv[k * SHARD:(k + 1) * SHARD]
        im.update({
            "xT": np.ascontiguousarray(xs.T).astype(bf16),
            "dinv_w": np.ascontiguousarray(dvk.reshape(W, 128).T),
            "lin1T": np.ascontiguousarray(lin1_w.T).astype(bf16),
            "phi1T": np.ascontiguousarray(phi1_w.T).astype(bf16),
            "aw1T": np.ascontiguousarray(
                (W1 - W1.T - GAMMA * np.eye(HID, dtype=np.float32)).T).astype(bf16),
            "lin2T": np.ascontiguousarray(lin2_w.T).astype(bf16),
            "phi2T": np.ascontiguousarray(phi2_w.T).astype(bf16),
            "aw2T": np.ascontiguousarray(
                (W2 - W2.T - GAMMA * np.eye(C, dtype=np.float32)).T).astype(bf16),
        })
        in_maps.append(im)

    biases = {
        "blin1": np.broadcast_to(lin1_b, (128, HID)).astype(np.float32).copy(),
        "bconv1": np.broadcast_to(b1, (128, HID)).astype(np.float32).copy(),
        "blin2": np.broadcast_to(lin2_b, (128, C)).astype(np.float32).copy(),
        "bconv2": np.broadcast_to(b2, (128, C)).astype(np.float32).copy(),
    }
    use_bias = {name: bool(np.any(arr)) for name, arr in biases.items()}
    for name, used in use_bias.items():
        if used:
            for im in in_maps:
                im[name] = biases[name]

    meta = {
        "LMAX": LMAX,
        "rows_q": rows_q, "ntiles_q": ntiles_q,
        "ops_q": ops_q, "calls_q": calls_q,
        "use_bias": use_bias,
    }
    return in_maps, meta


def _build_graph(meta):
    import concourse.bass as bass
    import concourse.mybir as mybir
    import concourse.tile as tile
    from concourse import bacc
    from concourse.masks import make_identity
    from contextlib import ExitStack

    dt = mybir.dt
    Alu = mybir.AluOpType
    Act = mybir.ActivationFunctionType
    rows_q = meta["rows_q"]
    ops_q = meta["ops_q"]
    calls_q = meta["calls_q"]
    use_bias = meta["use_bias"]

    nc = bacc.Bacc("TRN2", target_bir_lowering=False, num_swdge_queues=4,
                   dynamic_dma_scratch_size=16384)

    xT = nc.declare_dram_parameter("xT", [F_IN, PADN], dt.bfloat16, isOutput=False)
    dinv_w = nc.declare_dram_parameter("dinv_w", [128, W], dt.float32, isOutput=False)
    lin1T = nc.declare_dram_parameter("lin1T", [F_IN, HID], dt.bfloat16, isOutput=False)
    phi1T = nc.declare_dram_parameter("phi1T", [HID, HID], dt.bfloat16, isOutput=False)
    aw1T = nc.declare_dram_parameter("aw1T", [HID, HID], dt.bfloat16, isOutput=False)
    lin2T = nc.declare_dram_parameter("lin2T", [HID, C], dt.bfloat16, isOutput=False)
    phi2T = nc.declare_dram_parameter("phi2T", [C, C], dt.bfloat16, isOutput=False)
    aw2T = nc.declare_dram_parameter("aw2T", [C, C], dt.bfloat16, isOutput=False)
    idx_p, colv_p = [], []
    for q in range(4):
        nops = len(ops_q[q][0])
        idx_p.append(nc.declare_dram_parameter(
            f"idx{q}", [128, rows_q[q] // 16], dt.int16, isOutput=False))
        colv_p.append(nc.declare_dram_parameter(
            f"colv{q}", [128, 2 * nops], dt.bfloat16, isOutput=False))
    bias_p = {}
    for name, shape in [("blin1", [128, HID]), ("bconv1", [128, HID]),
                        ("blin2", [128, C]), ("bconv2", [128, C])]:
        if use_bias[name]:
            bias_p[name] = nc.declare_dram_parameter(name, shape, dt.float32,
                                                     isOutput=False)
    out_p = nc.declare_dram_parameter("out", [PADN, C], dt.float32, isOutput=True)

    t1q_in = [nc.dram_tensor(f"t1in{q}", [QROWS[q], HID], dt.bfloat16)
              for q in range(4)]
    t1q_tab = [nc.dram_tensor(f"t1tab{q}", [NCORES * QROWS[q], HID], dt.bfloat16,
                              addr_space="Shared") for q in range(4)]
    t2q_in = [nc.dram_tensor(f"t2in{q}", [QROWS[q], 128], dt.bfloat16)
              for q in range(4)]
    t2q_tab = [nc.dram_tensor(f"t2tab{q}", [NCORES * QROWS[q], 128], dt.bfloat16,
                              addr_space="Shared") for q in range(4)]

    rg = [list(range(NCORES))]

    with tile.TileContext(nc) as tc, ExitStack() as top:
        const = top.enter_context(tc.tile_pool(name="const", bufs=1))
        big = top.enter_context(tc.tile_pool(name="big", bufs=1))
        tmp_pool = top.enter_context(tc.tile_pool(name="tmp", bufs=3))

        lin1T_sb = const.tile([128, 2, HID], dt.bfloat16)
        nc.sync.dma_start(lin1T_sb[:], lin1T[:].rearrange("(t p) j -> p t j", p=128))
        phi1T_sb = const.tile([128, HID], dt.bfloat16)
        nc.sync.dma_start(phi1T_sb[:], phi1T[:])
        aw1T_sb = const.tile([128, HID], dt.bfloat16)
        nc.sync.dma_start(aw1T_sb[:], aw1T[:])
        lin2T_sb = const.tile([128, C], dt.bfloat16)
        nc.sync.dma_start(lin2T_sb[:], lin2T[:])
        phi2T_sb = const.tile([128, C], dt.bfloat16)
        aw2T_sb = const.tile([128, C], dt.bfloat16)
        for r in range(4):
            nc.sync.dma_start(phi2T_sb[r * C:(r + 1) * C, :], phi2T[:])
            nc.sync.dma_start(aw2T_sb[r * C:(r + 1) * C, :], aw2T[:])
        dinv_sb = const.tile([128, W], dt.float32)
        nc.sync.dma_start(dinv_sb[:], dinv_w[:])
        bias_sb = {}
        for name, p in bias_p.items():
            t = const.tile(list(p.shape), dt.float32)
            nc.sync.dma_start(t[:], p[:])
            bias_sb[name] = t

        iota_i = const.tile([128, 128], dt.int32)
        nc.gpsimd.iota(iota_i[:], pattern=[[1, 128]], base=0, channel_multiplier=0)
        iota_bf = const.tile([128, 128], dt.bfloat16)
        nc.vector.tensor_copy(iota_bf[:], iota_i[:])
        ident = const.tile([128, 128], dt.bfloat16)
        make_identity(nc, ident[:])

        # persistent state
        h1 = big.tile([128, W, HID], dt.bfloat16)
        h1T_all = big.tile([128, W, HID], dt.bfloat16, tag="h1T_all")
        agg = big.tile([128, W, HID], dt.bfloat16, tag="agg")        # L1 carrier
        h2 = big.tile([128, W, C], dt.bfloat16, tag="h2")
        agg2 = big.tile([128, W, C], dt.bfloat16, tag="agg2")        # L2 carrier
        h2T_all = big.tile([128, (W + 1) // 2, 128], dt.bfloat16, tag="h2T_all")

        SOPS = max(o1 - o0 for q in range(4)
                   for o0, o1 in _call_op_ranges(ops_q[q], calls_q[q]))

        with tc.tile_pool(name="gp", bufs=8) as gp, \
             tc.tile_pool(name="sp", bufs=3) as sp, \
             tc.tile_pool(name="ip", bufs=1) as ip, \
             tc.tile_pool(name="cp", bufs=1) as cp, \
             tc.tile_pool(name="psC", bufs=4, space="PSUM") as psC, \
             ExitStack() as stA, ExitStack() as stD:

            xqp = stA.enter_context(tc.tile_pool(name="xq", bufs=1))
            psA = stA.enter_context(tc.tile_pool(name="psA", bufs=1,
                                                 space="PSUM"))
            psAT = stA.enter_context(tc.tile_pool(name="psAT", bufs=1,
                                                  space="PSUM"))

            # ===== Phase A: h1, h1T, T1 (self term into agg) + AllGather =====
            for q in range(4):
                for wi0 in range(0, QW[q], 13):
                    wn = min(13, QW[q] - wi0)
                    c0 = (QWSTART[q] + wi0) * 128
                    cw = wn * 128
                    xq_sb = xqp.tile([128, 2, 13 * 128], dt.bfloat16, tag="xq")
                    nc.sync.dma_start(
                        xq_sb[:, :, 0:cw],
                        xT[:, c0:c0 + cw].rearrange("(t p) c -> p t c", p=128))
                    for wi in range(wi0, wi0 + wn):
                        w = QWSTART[q] + wi
                        j = wi - wi0
                        ph = psA.tile([128, HID], dt.float32, tag="ph")
                        nc.tensor.matmul(ph[:], xq_sb[:, 0, j * 128:(j + 1) * 128],
                                         lin1T_sb[:, 0, :], start=True, stop=False)
                        nc.tensor.matmul(ph[:], xq_sb[:, 1, j * 128:(j + 1) * 128],
                                         lin1T_sb[:, 1, :], start=False, stop=True)
                        if "blin1" in bias_sb:
                            t = tmp_pool.tile([128, HID], dt.float32, tag="tA")
                            nc.vector.tensor_tensor(t[:], ph[:],
                                                    bias_sb["blin1"][:],
                                                    op=Alu.add)
                            nc.scalar.activation(h1[:, w, :], t[:], Act.Relu)
                        else:
                            nc.scalar.activation(h1[:, w, :], ph[:], Act.Relu)
                        pt = psAT.tile([128, 128], dt.bfloat16, tag="pt")
                        nc.tensor.transpose(pt[:], h1[:, w, :], ident[:])
                        nc.scalar.copy(h1T_all[:, w, :], pt[:])
                        pT = psA.tile([128, HID], dt.float32, tag="pT1")
                        nc.tensor.matmul(pT[:], h1T_all[:, w, :], phi1T_sb[:],
                                         start=True, stop=True)
                        nc.scalar.activation(agg[:, w, :], pT[:], Act.Copy,
                                             scale=dinv_sb[:, w:w + 1])
                nc.sync.dma_start(
                    t1q_in[q][:].rearrange("(w p) f -> p w f", p=128),
                    agg[:, QWSTART[q]:QWSTART[q] + QW[q], :])
                nc.gpsimd.collective_compute(
                    "AllGather", Alu.bypass, replica_groups=rg,
                    ins=[t1q_in[q][:].opt()], outs=[t1q_tab[q][:].opt()])

            stA.close()

            # ===== aggregation pass over one quarter's stream =====
            qctr = [0]

            def aggregate(table, fw, agg_t, q, post_cb=None):
                op_t, op_w, op_st, op_sp = ops_q[q]
                ranges = _call_op_ranges(ops_q[q], calls_q[q])
                colv_sb = cp.tile([128, 2 * len(op_t)], dt.bfloat16, tag="colv")
                nc.sync.dma_start(colv_sb[:], colv_p[q][:])
                idx_sb = ip.tile([128, max(rows_q) // 16], dt.int16, tag="idx")
                nc.sync.dma_start(idx_sb[:, 0:rows_q[q] // 16], idx_p[q][:])
                pseg = {}
                for ci, (t0, nt) in enumerate(calls_q[q]):
                    rows = nt * 128
                    g = gp.tile([128, CALL_TILES, 128], dt.bfloat16, tag="g")
                    nc.gpsimd.dma_gather(
                        g[:, 0:nt, :], table[q][:],
                        idx_sb[:, t0 * 8:t0 * 8 + nt * 8],
                        rows, rows, 128, queue_num=qctr[0] % 4,
                        single_packet=False)
                    qctr[0] += 1
                    o0, o1 = ranges[ci]
                    no = o1 - o0
                    S = sp.tile([128, SOPS, 128], dt.bfloat16, tag="S")
                    # every operand has inner step 1 (DVE 2x_1P mode):
                    # S[p,o,h,l] = (iota[h,l] == colv2[p,o,l])
                    nc.vector.tensor_tensor(
                        S[:, 0:no, :].rearrange("p o (h l) -> p o h l", l=2),
                        iota_bf[:].rearrange("p (h l) -> p h l", l=2)
                            .unsqueeze(1).broadcast_to([128, no, 64, 2]),
                        colv_sb[:, 2 * o0:2 * o1]
                            .rearrange("p (o l) -> p o l", l=2)
                            .unsqueeze(2).broadcast_to([128, no, 64, 2]),
                        op=Alu.is_equal)
                    for o in range(o0, o1):
                        w = int(op_w[o])
                        grp = w // GRP
                        slot = w % GRP
                        if op_st[o]:
                            if grp not in pseg:
                                pseg[grp] = psC.tile([128, GRP, fw], dt.float32,
                                                     name="pseg", tag="pseg")
                            nc.tensor.matmul(pseg[grp][:, slot, :], ident[:],
                                             agg_t[:, w, 0:fw],
                                             start=True, stop=False)
                        nc.tensor.matmul(pseg[grp][:, slot, :],
                                         S[:, o - o0, :],
                                         g[:, int(op_t[o]) - t0, 0:fw],
                                         start=False, stop=bool(op_sp[o]))
                        if op_sp[o] and (slot == GRP - 1 or w == W - 1):
                            w0 = grp * GRP
                            gn = w - w0 + 1
                            nc.scalar.copy(agg_t[:, w0:w0 + gn, :],
                                           pseg[grp][:, 0:gn, :])
                            del pseg[grp]
                            if post_cb is not None:
                                post_cb(w0, gn)

            # ===== Phase C: layer-1 aggregation, passes 0..2 =====
            for q in range(3):
                aggregate(t1q_tab, HID, agg, q)

            # ===== Phase D machinery (runs in pass-3 callbacks) =====
            t2qbp = stD.enter_context(tc.tile_pool(name="t2qb", bufs=2))
            dstg = stD.enter_context(tc.tile_pool(name="dstg", bufs=2))
            psD = stD.enter_context(tc.tile_pool(name="psD", bufs=1,
                                                 space="PSUM"))
            psDt = stD.enter_context(tc.tile_pool(name="psDt", bufs=1,
                                                  space="PSUM"))
            # zero both rotating t2qb bufs once; cols C:128 stay zero
            for _ in range(2):
                z = t2qbp.tile([128, 25, 128], dt.bfloat16, tag="t2qb")
                nc.vector.memset(z[:], 0.0)

            dstate = {}

            def d_window(w):
                Q = 0 if w < 25 else (1 if w < 50 else (2 if w < 74 else 3))
                if dstate.get("Q") != Q:
                    dstate["Q"] = Q
                    dstate["t2qb"] = t2qbp.tile([128, 25, 128], dt.bfloat16,
                                                name="t2qb", tag="t2qb")
                i = w - QWSTART[Q]
                # s1: h1' = h1 + eps*tanh(aw@.. + dinv*agg)
                paw = psD.tile([128, HID], dt.float32, tag="paw")
                nc.tensor.matmul(paw[:], h1T_all[:, w, :], aw1T_sb[:],
                                 start=True, stop=True)
                pre = tmp_pool.tile([128, HID], dt.float32, tag="pre")
                nc.vector.scalar_tensor_tensor(
                    pre[:], agg[:, w, :], dinv_sb[:, w:w + 1], paw[:],
                    op0=Alu.mult, op1=Alu.add)
                if "bconv1" in bias_sb:
                    nc.vector.tensor_tensor(
                        pre[:], pre[:], bias_sb["bconv1"][:], op=Alu.add)
                th = tmp_pool.tile([128, HID], dt.float32, tag="th")
                nc.scalar.activation(th[:], pre[:], Act.Tanh)
                h1p = dstg.tile([128, HID], dt.bfloat16, tag="h1p")
                nc.vector.scalar_tensor_tensor(
                    h1p[:], th[:], EPS, h1[:, w, :],
                    op0=Alu.mult, op1=Alu.add)
                # s2: h2 = h1' @ lin2
                pt2 = psDt.tile([128, 128], dt.bfloat16, tag="ptD")
                nc.tensor.transpose(pt2[:], h1p[:], ident[:])
                h1pT = tmp_pool.tile([128, 128], dt.bfloat16, tag="h1pT")
                nc.scalar.copy(h1pT[:], pt2[:])
                ph2 = psD.tile([128, C], dt.float32, tag="pD32")
                nc.tensor.matmul(ph2[:], h1pT[:], lin2T_sb[:],
                                 start=True, stop=True)
                if "blin2" in bias_sb:
                    nc.vector.tensor_tensor(
                        h2[:, w, :], ph2[:], bias_sb["blin2"][:], op=Alu.add)
                else:
                    nc.scalar.copy(h2[:, w, :], ph2[:])
                # s3: h2T, T2 = dinv * (h2 @ phi2^T); agg2 self term
                p0 = (w % 2) * 64
                pt3 = psDt.tile([C, 128], dt.bfloat16, tag="pt3")
                nc.tensor.transpose(pt3[:], h2[:, w, :], ident[:])
                nc.scalar.copy(h2T_all[p0:p0 + C, w // 2, :], pt3[:])
                pT2 = psD.tile([128, C], dt.float32, tag="pD32")
                nc.tensor.matmul(pT2[:], h2T_all[p0:p0 + C, w // 2, :],
                                 phi2T_sb[p0:p0 + C, :],
                                 start=True, stop=True)
                nc.scalar.activation(dstate["t2qb"][:, i, 0:C], pT2[:],
                                     Act.Copy, scale=dinv_sb[:, w:w + 1])
                nc.scalar.activation(agg2[:, w, :], pT2[:],
                                     Act.Copy, scale=dinv_sb[:, w:w + 1])
                if w == QWEND[Q]:
                    nc.sync.dma_start(
                        t2q_in[Q][:].rearrange("(w p) f -> p w f", p=128),
                        dstate["t2qb"][:, 0:QW[Q], :])
                    nc.gpsimd.collective_compute(
                        "AllGather", Alu.bypass, replica_groups=rg,
                        ins=[t2q_in[Q][:].opt()], outs=[t2q_tab[Q][:].opt()])

            def d_cb(w0, gn):
                for i in range(gn):
                    d_window(w0 + i)

            # layer-1 pass 3 with phase-D callbacks
            aggregate(t1q_tab, HID, agg, 3, post_cb=d_cb)

            # ===== Phase G (runs in layer-2 pass-3 callbacks) =====
            def g_group(w0, gw):
                a1 = tmp_pool.tile([128, GRP, C], dt.float32, tag="a1g")
                nc.vector.tensor_tensor(
                    a1[:, 0:gw, :], agg2[:, w0:w0 + gw, :],
                    dinv_sb[:, w0:w0 + gw].unsqueeze(2)
                        .broadcast_to([128, gw, C]),
                    op=Alu.mult)
                pre = tmp_pool.tile([128, GRP, C], dt.float32, tag="preg")
                for wi in range(gw):
                    w = w0 + wi
                    p0 = (w % 2) * 64
                    pawt = psD.tile([128, C], dt.float32, tag="pD32")
                    nc.tensor.matmul(pawt[:],
                                     h2T_all[p0:p0 + C, w // 2, :],
                                     aw2T_sb[p0:p0 + C, :],
                                     start=True, stop=True)
                    nc.vector.tensor_tensor(
                        pre[:, wi, :], a1[:, wi, :], pawt[:], op=Alu.add)
                if "bconv2" in bias_sb:
                    nc.vector.tensor_tensor(
                        pre[:, 0:gw, :], pre[:, 0:gw, :],
                        bias_sb["bconv2"][:].unsqueeze(1)
                            .broadcast_to([128, gw, C]),
                        op=Alu.add)
                th = tmp_pool.tile([128, GRP, C], dt.float32, tag="thg")
                nc.scalar.activation(th[:, 0:gw, :], pre[:, 0:gw, :], Act.Tanh)
                h2p = tmp_pool.tile([128, GRP, C], dt.float32, tag="h2pg")
                nc.vector.scalar_tensor_tensor(
                    h2p[:, 0:gw, :], th[:, 0:gw, :], EPS,
                    h2[:, w0:w0 + gw, :], op0=Alu.mult, op1=Alu.add)
                negmax = tmp_pool.tile([128, GRP, 1], dt.float32, tag="nmg")
                nc.vector.tensor_reduce(negmax[:, 0:gw, :], h2p[:, 0:gw, :],
                                        axis=mybir.AxisListType.X,
                                        op=Alu.max, negate=True)
                sub = tmp_pool.tile([128, GRP, C], dt.float32, tag="subg")
                nc.vector.tensor_tensor(
                    sub[:, 0:gw, :], h2p[:, 0:gw, :],
                    negmax[:, 0:gw, :].broadcast_to([128, gw, C]),
                    op=Alu.add)
                e = tmp_pool.tile([128, GRP, C], dt.float32, tag="eg")
                nc.scalar.activation(e[:, 0:gw, :], sub[:, 0:gw, :], Act.Exp)
                ssum = tmp_pool.tile([128, GRP, 1], dt.float32, tag="ssg")
                nc.vector.tensor_reduce(ssum[:, 0:gw, :], e[:, 0:gw, :],
                                        axis=mybir.AxisListType.X,
                                        op=Alu.add)
                lse = tmp_pool.tile([128, GRP, 1], dt.float32, tag="lseg")
                nc.scalar.activation(lse[:, 0:gw, :], ssum[:, 0:gw, :], Act.Ln)
                fin = tmp_pool.tile([128, GRP, C], dt.float32, tag="fing")
                nc.vector.tensor_tensor(
                    fin[:, 0:gw, :], sub[:, 0:gw, :],
                    lse[:, 0:gw, :].broadcast_to([128, gw, C]),
                    op=Alu.subtract)
                nc.sync.dma_start(
                    out_p[w0 * 128:(w0 + gw) * 128, :]
                        .rearrange("(w p) c -> p w c", p=128),
                    fin[:, 0:gw, :])

            # ===== Phase F: layer-2 aggregation =====
            for q in range(4):
                aggregate(t2q_tab, C, agg2, q,
                          post_cb=g_group if q == 3 else None)

    nc.compile()
    return nc


def _call_op_ranges(ops, calls):
    """[o0, o1) op index range per gather call (ops sorted by tile)."""
    op_t = ops[0]
    ranges = []
    for t0, nt in calls:
        o0 = int(np.searchsorted(op_t, t0, side="left"))
        o1 = int(np.searchsorted(op_t, t0 + nt - 1, side="right"))
        ranges.append((o0, o1))
    return ranges


def kernel(**inputs):
    from concourse.bass_utils import run_bass_kernel_spmd

    inp = {k: np.asarray(v) for k, v in inputs.items()}
    in_maps, meta = _host_prep(**inp)

    key = ("graph", tuple(meta["LMAX"].tolist()),
           tuple(sorted(meta["use_bias"].items())))
    if key not in _CACHE:
        _CACHE[key] = _build_graph(meta)
    nc = _CACHE[key]

    import os
    trace = bool(int(os.environ.get("KERNEL_TRACE", "0")))
    res = run_bass_kernel_spmd(nc, in_maps, list(range(NCORES)), trace=trace,
                               tmpdir=os.environ.get("KERNEL_TRACE_DIR"))
    global LAST_EXEC_NS
    LAST_EXEC_NS = res.exec_time_ns

    out = np.concatenate([res.results[k]["out"][:SHARD] for k in range(NCORES)], 0)
    return out.astype(np.float32)


LAST_EXEC_NS = None


# revision 30
# speedup vs baseline: 1.3066x; 1.1297x over previous
"""AntiSymmetric GNN (2x AntiSymmetricConv + linear layers + log_softmax)
distributed Bass kernel for 8 TRN2 NeuronCores.

Strategy (v2):
  - Nodes sharded by destination across 8 cores (12500/core, padded 12544).
  - Edges partitioned by destination core; per core bucketed into
    (source-quarter, dest-window) segments, padded to the cross-core max
    (LMAX) with cycled real indices so the instruction stream is SPMD.
  - Per quarter the 98 window segments are CONCATENATED into one stream;
    dma_gather runs in large ~CALL_TILES*128-row calls (ring allows
    num_idxs/16+1 descs of 1024), cutting the ~1.2us/call Q7 fixed cost
    ~20x vs per-window gathers. Calls rotate the 4 SWDGE queues.
  - 128-row tiles of the stream may span window boundaries; the host emits
    a static op list (tile, window, colv column, start, stop); the one-hot
    scatter matrices S are built by one batched is_equal per call against
    a per-op colv table (rows outside the op's window get -1 -> S row 0).
  - Aggregation across the 4 quarter passes uses a bf16 agg carrier in
    SBUF: each window chain begins with an identity-matmul "fold" of the
    carrier into PSUM and ends with a scalar-engine drain (4 windows per
    PSUM bank) back to the carrier.  No vector-engine adds at all.
  - gcn norm factorizes: agg[c] = dinv[c] * (sum_e T[src_e] + T[c]),
    T = dinv*xw; phase A writes the self-loop term T[c] into the carrier.
  - Both layers' tables are 256B bf16 rows ([*,HID] and [*,128] zero
    padded) so one gather path serves both layers.
  - Tables are AllGathered in 4 window-aligned chunks (<=25600 rows so
    gather indices fit int16); layer-2 table chunks are produced by the
    phase-D callbacks embedded in layer-1's last pass, and the final
    conv+log_softmax runs in callbacks of layer-2's last pass.
"""

import numpy as np
import ml_dtypes

N = 100_000
F_IN = 256
HID = 128
C = 32
EPS = 0.1
GAMMA = 0.1

NCORES = 8
SHARD = 12_500
PADN = 12_544            # 98 * 128
W = 98                   # windows per core
QW = [25, 25, 24, 24]    # windows per quarter chunk (table chunks)
QROWS = [3200, 3200, 3072, 3072]
QSTART = [0, 3200, 6400, 9472]
QWSTART = [0, 25, 50, 74]
QWEND = [24, 49, 73, 97]  # last window of each t2-quarter chunk

CALL_TILES = 12          # tiles (128 rows) per dma_gather call
GRP = 4                  # windows per PSUM drain group

_CACHE = {}


def _host_prep(x, lin1_w, lin1_b, lin2_w, lin2_b, W1, phi1_w, b1, W2, phi2_w, b2,
               edge_index):
    bf16 = ml_dtypes.bfloat16
    row = edge_index[0].astype(np.int64)
    col = edge_index[1].astype(np.int64)

    # degrees INCLUDE self loops (reference appends them)
    deg = (np.bincount(col, minlength=N) + 1).astype(np.float32)
    dinv = 1.0 / np.sqrt(deg)

    # source -> (quarter chunk, int16 index into chunk table)
    ks = row // SHARD
    i_s = row % SHARD
    wloc = i_s // 128
    q_s = np.where(wloc < 25, 0, np.where(wloc < 50, 1, np.where(wloc < 74, 2, 3)))
    pos = i_s - np.asarray(QSTART)[q_s]
    idx16_all = ks * np.asarray(QROWS)[q_s] + pos

    k_dst = col // SHARD

    cores = []
    L = np.zeros((NCORES, 4 * W), np.int64)
    for k in range(NCORES):
        m = k_dst == k
        r_idx = idx16_all[m]
        c_loc = col[m] - k * SHARD
        key = q_s[m] * W + c_loc // 128
        # within each segment, order by table row so gather descriptors
        # hit ascending HBM addresses (DRAM locality)
        order = np.lexsort((r_idx, key))
        cores.append((key[order], r_idx[order],
                      (c_loc % 128)[order].astype(np.float32)))
        L[k] = np.bincount(cores[k][0], minlength=4 * W)

    # every (quarter, window) keeps >=1 row so every window has an op in
    # every pass (keeps the fold/drain grouping uniform)
    LMAX = np.maximum(L.max(axis=0), 1)

    # ---- shared (LMAX-derived) stream layout + op lists per quarter ----
    seg_start = np.zeros((4, W), np.int64)
    rows_q = []          # padded rows per quarter
    ntiles_q = []
    ops_q = []           # per q: (op_tile, op_w, op_start, op_stop) arrays
    calls_q = []         # per q: list of (t0, nt)
    row_window = []      # per q: [rows] window id of each stream row
    for q in range(4):
        off = 0
        rw = []
        for w in range(W):
            seg_start[q, w] = off
            lm = int(LMAX[q * W + w])
            rw.append(np.full(lm, w, np.int32))
            off += lm
        rows_pad = -(-off // 128) * 128
        rw.append(np.full(rows_pad - off, -1, np.int32))
        rwin = np.concatenate(rw)
        nt = rows_pad // 128
        ot, ow, ost, osp = [], [], [], []
        for w in range(W):
            a = int(seg_start[q, w])
            b = a + int(LMAX[q * W + w])
            t0, t1 = a // 128, (b - 1) // 128
            for t in range(t0, t1 + 1):
                ot.append(t)
                ow.append(w)
                ost.append(t == t0)
                osp.append(t == t1)
        rows_q.append(rows_pad)
        ntiles_q.append(nt)
        ops_q.append((np.asarray(ot), np.asarray(ow),
                      np.asarray(ost), np.asarray(osp)))
        calls_q.append([(t0, min(CALL_TILES, nt - t0))
                        for t0 in range(0, nt, CALL_TILES)])
        row_window.append(rwin)

    def wrap_idx(arr):
        a16 = arr.reshape(-1, 16).T
        return np.ascontiguousarray(np.tile(a16, (8, 1)))

    # ---- per-core data ----
    in_maps = []
    for k in range(NCORES):
        key_s, idx_s, cl_s = cores[k]
        starts_src = np.zeros(4 * W + 1, np.int64)
        np.cumsum(L[k], out=starts_src[1:])
        im = {}
        for q in range(4):
            rows_pad = rows_q[q]
            idx_arr = np.zeros(rows_pad, np.int16)
            rcolv = np.full(rows_pad, -1.0, np.float32)
            for w in range(W):
                s = q * W + w
                a = int(seg_start[q, w])
                lk = int(L[k][s])
                lm = int(LMAX[s])
                if lk > 0:
                    src0 = int(starts_src[s])
                    seg_idx = idx_s[src0:src0 + lk].astype(np.int16)
                    idx_arr[a:a + lk] = seg_idx
                    rcolv[a:a + lk] = cl_s[src0:src0 + lk]
                    if lm > lk:
                        idx_arr[a + lk:a + lm] = np.resize(seg_idx, lm - lk)
            ops_t, ops_w, _, _ = ops_q[q]
            tile_rows = ops_t[:, None] * 128 + np.arange(128)[None, :]
            cm = np.where(row_window[q][tile_rows] == ops_w[:, None],
                          rcolv[tile_rows], -1.0)           # [nops, 128]
            # duplicated x2 so the kernel's one-hot compare has inner step 1
            # on every operand (DVE 2x mode requires it)
            cm2 = np.repeat(cm.T.astype(bf16)[:, :, None], 2, axis=2)
            im[f"idx{q}"] = wrap_idx(idx_arr)
            im[f"colv{q}"] = np.ascontiguousarray(cm2.reshape(128, -1))

        xs = np.zeros((PADN, F_IN), np.float32)
        xs[:SHARD] = x[k * SHARD:(k + 1) * SHARD]
        dvk = np.zeros(PADN, np.float32)
        dvk[:SHARD] = dinv[k * SHARD:(k + 1) * SHARD]
        im.update({
            "xT": np.ascontiguousarray(xs.T).astype(bf16),
            "dinv_w": np.ascontiguousarray(dvk.reshape(W, 128).T),
            "lin1T": np.ascontiguousarray(lin1_w.T).astype(bf16),
            "phi1T": np.ascontiguousarray(phi1_w.T).astype(bf16),
            "aw1T": np.ascontiguousarray(
                (W1 - W1.T - GAMMA * np.eye(HID, dtype=np.float32)).T).astype(bf16),
            "lin2T": np.ascontiguousarray(lin2_w.T).astype(bf16),
            "phi2T": np.ascontiguousarray(phi2_w.T).astype(bf16),
            "aw2T": np.ascontiguousarray(
                (W2 - W2.T - GAMMA * np.eye(C, dtype=np.float32)).T).astype(bf16),
        })
        in_maps.append(im)

    biases = {
        "blin1": np.broadcast_to(lin1_b, (128, HID)).astype(np.float32).copy(),
        "bconv1": np.broadcast_to(b1, (128, HID)).astype(np.float32).copy(),
        "blin2": np.broadcast_to(lin2_b, (128, C)).astype(np.float32).copy(),
        "bconv2": np.broadcast_to(b2, (128, C)).astype(np.float32).copy(),
    }
    use_bias = {name: bool(np.any(arr)) for name, arr in biases.items()}
    for name, used in use_bias.items():
        if used:
            for im in in_maps:
                im[name] = biases[name]

    meta = {
        "LMAX": LMAX,
        "rows_q": rows_q, "ntiles_q": ntiles_q,
        "ops_q": ops_q, "calls_q": calls_q,
        "use_bias": use_bias,
    }
    return in_maps, meta


def _build_graph(meta):
    import concourse.bass as bass
    import concourse.mybir as mybir
    import concourse.tile as tile
    from concourse import bacc
    from concourse.masks import make_identity
    from contextlib import ExitStack

    dt = mybir.dt
    Alu = mybir.AluOpType
    Act = mybir.ActivationFunctionType
    rows_q = meta["rows_q"]
    ops_q = meta["ops_q"]
    calls_q = meta["calls_q"]
    use_bias = meta["use_bias"]

    nc = bacc.Bacc("TRN2", target_bir_lowering=False, num_swdge_queues=4,
                   dynamic_dma_scratch_size=16384)

    xT = nc.declare_dram_parameter("xT", [F_IN, PADN], dt.bfloat16, isOutput=False)
    dinv_w = nc.declare_dram_parameter("dinv_w", [128, W], dt.float32, isOutput=False)
    lin1T = nc.declare_dram_parameter("lin1T", [F_IN, HID], dt.bfloat16, isOutput=False)
    phi1T = nc.declare_dram_parameter("phi1T", [HID, HID], dt.bfloat16, isOutput=False)
    aw1T = nc.declare_dram_parameter("aw1T", [HID, HID], dt.bfloat16, isOutput=False)
    lin2T = nc.declare_dram_parameter("lin2T", [HID, C], dt.bfloat16, isOutput=False)
    phi2T = nc.declare_dram_parameter("phi2T", [C, C], dt.bfloat16, isOutput=False)
    aw2T = nc.declare_dram_parameter("aw2T", [C, C], dt.bfloat16, isOutput=False)
    idx_p, colv_p = [], []
    for q in range(4):
        nops = len(ops_q[q][0])
        idx_p.append(nc.declare_dram_parameter(
            f"idx{q}", [128, rows_q[q] // 16], dt.int16, isOutput=False))
        colv_p.append(nc.declare_dram_parameter(
            f"colv{q}", [128, 2 * nops], dt.bfloat16, isOutput=False))
    bias_p = {}
    for name, shape in [("blin1", [128, HID]), ("bconv1", [128, HID]),
                        ("blin2", [128, C]), ("bconv2", [128, C])]:
        if use_bias[name]:
            bias_p[name] = nc.declare_dram_parameter(name, shape, dt.float32,
                                                     isOutput=False)
    out_p = nc.declare_dram_parameter("out", [PADN, C], dt.float32, isOutput=True)

    t1q_in = [nc.dram_tensor(f"t1in{q}", [QROWS[q], HID], dt.bfloat16)
              for q in range(4)]
    t1q_tab = [nc.dram_tensor(f"t1tab{q}", [NCORES * QROWS[q], HID], dt.bfloat16,
                              addr_space="Shared") for q in range(4)]
    t2q_in = [nc.dram_tensor(f"t2in{q}", [QROWS[q], 128], dt.bfloat16)
              for q in range(4)]
    t2q_tab = [nc.dram_tensor(f"t2tab{q}", [NCORES * QROWS[q], 128], dt.bfloat16,
                              addr_space="Shared") for q in range(4)]

    rg = [list(range(NCORES))]

    with tile.TileContext(nc) as tc, ExitStack() as top:
        const = top.enter_context(tc.tile_pool(name="const", bufs=1))
        big = top.enter_context(tc.tile_pool(name="big", bufs=1))
        tmp_pool = top.enter_context(tc.tile_pool(name="tmp", bufs=3))

        lin1T_sb = const.tile([128, 2, HID], dt.bfloat16)
        nc.sync.dma_start(lin1T_sb[:], lin1T[:].rearrange("(t p) j -> p t j", p=128))
        phi1T_sb = const.tile([128, HID], dt.bfloat16)
        nc.sync.dma_start(phi1T_sb[:], phi1T[:])
        aw1T_sb = const.tile([128, HID], dt.bfloat16)
        nc.sync.dma_start(aw1T_sb[:], aw1T[:])
        lin2T_sb = const.tile([128, C], dt.bfloat16)
        nc.sync.dma_start(lin2T_sb[:], lin2T[:])
        phi2T_sb = const.tile([128, C], dt.bfloat16)
        aw2T_sb = const.tile([128, C], dt.bfloat16)
        for r in range(4):
            nc.sync.dma_start(phi2T_sb[r * C:(r + 1) * C, :], phi2T[:])
            nc.sync.dma_start(aw2T_sb[r * C:(r + 1) * C, :], aw2T[:])
        dinv_sb = const.tile([128, W], dt.float32)
        nc.sync.dma_start(dinv_sb[:], dinv_w[:])
        bias_sb = {}
        for name, p in bias_p.items():
            t = const.tile(list(p.shape), dt.float32)
            nc.sync.dma_start(t[:], p[:])
            bias_sb[name] = t

        iota_i = const.tile([128, 128], dt.int32)
        nc.gpsimd.iota(iota_i[:], pattern=[[1, 128]], base=0, channel_multiplier=0)
        iota_bf = const.tile([128, 128], dt.bfloat16)
        nc.vector.tensor_copy(iota_bf[:], iota_i[:])
        ident = const.tile([128, 128], dt.bfloat16)
        make_identity(nc, ident[:])

        # persistent state
        h1 = big.tile([128, W, HID], dt.bfloat16)
        h1T_all = big.tile([128, W, HID], dt.bfloat16, tag="h1T_all")
        agg = big.tile([128, W, HID], dt.bfloat16, tag="agg")        # L1 carrier
        h2 = big.tile([128, W, C], dt.bfloat16, tag="h2")
        agg2 = big.tile([128, W, C], dt.bfloat16, tag="agg2")        # L2 carrier
        h2T_all = big.tile([128, (W + 1) // 2, 128], dt.bfloat16, tag="h2T_all")

        SOPS = max(o1 - o0 for q in range(4)
                   for o0, o1 in _call_op_ranges(ops_q[q], calls_q[q]))

        with tc.tile_pool(name="gp", bufs=13) as gp, \
             tc.tile_pool(name="sp", bufs=3) as sp, \
             tc.tile_pool(name="ip", bufs=2) as ip, \
             tc.tile_pool(name="cp", bufs=2) as cp, \
             tc.tile_pool(name="psC", bufs=4, space="PSUM") as psC, \
             ExitStack() as stA, ExitStack() as stD:

            xqp = stA.enter_context(tc.tile_pool(name="xq", bufs=1))
            psA = stA.enter_context(tc.tile_pool(name="psA", bufs=1,
                                                 space="PSUM"))
            psAT = stA.enter_context(tc.tile_pool(name="psAT", bufs=1,
                                                  space="PSUM"))

            # ===== Phase A: h1, h1T, T1 (self term into agg) + AllGather =====
            for q in range(4):
                for wi0 in range(0, QW[q], 13):
                    wn = min(13, QW[q] - wi0)
                    c0 = (QWSTART[q] + wi0) * 128
                    cw = wn * 128
                    xq_sb = xqp.tile([128, 2, 13 * 128], dt.bfloat16, tag="xq")
                    nc.sync.dma_start(
                        xq_sb[:, :, 0:cw],
                        xT[:, c0:c0 + cw].rearrange("(t p) c -> p t c", p=128))
                    for wi in range(wi0, wi0 + wn):
                        w = QWSTART[q] + wi
                        j = wi - wi0
                        ph = psA.tile([128, HID], dt.float32, tag="ph")
                        nc.tensor.matmul(ph[:], xq_sb[:, 0, j * 128:(j + 1) * 128],
                                         lin1T_sb[:, 0, :], start=True, stop=False)
                        nc.tensor.matmul(ph[:], xq_sb[:, 1, j * 128:(j + 1) * 128],
                                         lin1T_sb[:, 1, :], start=False, stop=True)
                        if "blin1" in bias_sb:
                            t = tmp_pool.tile([128, HID], dt.float32, tag="tA")
                            nc.vector.tensor_tensor(t[:], ph[:],
                                                    bias_sb["blin1"][:],
                                                    op=Alu.add)
                            nc.scalar.activation(h1[:, w, :], t[:], Act.Relu)
                        else:
                            nc.scalar.activation(h1[:, w, :], ph[:], Act.Relu)
                        pt = psAT.tile([128, 128], dt.bfloat16, tag="pt")
                        nc.tensor.transpose(pt[:], h1[:, w, :], ident[:])
                        nc.scalar.copy(h1T_all[:, w, :], pt[:])
                        pT = psA.tile([128, HID], dt.float32, tag="pT1")
                        nc.tensor.matmul(pT[:], h1T_all[:, w, :], phi1T_sb[:],
                                         start=True, stop=True)
                        nc.scalar.activation(agg[:, w, :], pT[:], Act.Copy,
                                             scale=dinv_sb[:, w:w + 1])
                nc.sync.dma_start(
                    t1q_in[q][:].rearrange("(w p) f -> p w f", p=128),
                    agg[:, QWSTART[q]:QWSTART[q] + QW[q], :])
                nc.gpsimd.collective_compute(
                    "AllGather", Alu.bypass, replica_groups=rg,
                    ins=[t1q_in[q][:].opt()], outs=[t1q_tab[q][:].opt()])

            stA.close()

            # ===== aggregation pass over one quarter's stream =====
            qctr = [0]

            def aggregate(table, fw, agg_t, q, post_cb=None):
                op_t, op_w, op_st, op_sp = ops_q[q]
                ranges = _call_op_ranges(ops_q[q], calls_q[q])
                colv_sb = cp.tile([128, 2 * len(op_t)], dt.bfloat16, tag="colv")
                nc.sync.dma_start(colv_sb[:], colv_p[q][:])
                idx_sb = ip.tile([128, max(rows_q) // 16], dt.int16, tag="idx")
                nc.sync.dma_start(idx_sb[:, 0:rows_q[q] // 16], idx_p[q][:])
                pseg = {}
                for ci, (t0, nt) in enumerate(calls_q[q]):
                    rows = nt * 128
                    g = gp.tile([128, CALL_TILES, 128], dt.bfloat16, tag="g")
                    nc.gpsimd.dma_gather(
                        g[:, 0:nt, :], table[q][:],
                        idx_sb[:, t0 * 8:t0 * 8 + nt * 8],
                        rows, rows, 128, queue_num=qctr[0] % 4,
                        single_packet=False)
                    qctr[0] += 1
                    o0, o1 = ranges[ci]
                    no = o1 - o0
                    S = sp.tile([128, SOPS, 128], dt.bfloat16, tag="S")
                    # every operand has inner step 1 (DVE 2x_1P mode):
                    # S[p,o,h,l] = (iota[h,l] == colv2[p,o,l])
                    nc.vector.tensor_tensor(
                        S[:, 0:no, :].rearrange("p o (h l) -> p o h l", l=2),
                        iota_bf[:].rearrange("p (h l) -> p h l", l=2)
                            .unsqueeze(1).broadcast_to([128, no, 64, 2]),
                        colv_sb[:, 2 * o0:2 * o1]
                            .rearrange("p (o l) -> p o l", l=2)
                            .unsqueeze(2).broadcast_to([128, no, 64, 2]),
                        op=Alu.is_equal)
                    for o in range(o0, o1):
                        w = int(op_w[o])
                        grp = w // GRP
                        slot = w % GRP
                        if op_st[o]:
                            if grp not in pseg:
                                pseg[grp] = psC.tile([128, GRP, fw], dt.float32,
                                                     name="pseg", tag="pseg")
                            nc.tensor.matmul(pseg[grp][:, slot, :], ident[:],
                                             agg_t[:, w, 0:fw],
                                             start=True, stop=False)
                        nc.tensor.matmul(pseg[grp][:, slot, :],
                                         S[:, o - o0, :],
                                         g[:, int(op_t[o]) - t0, 0:fw],
                                         start=False, stop=bool(op_sp[o]))
                        if op_sp[o] and (slot == GRP - 1 or w == W - 1):
                            w0 = grp * GRP
                            gn = w - w0 + 1
                            nc.scalar.copy(agg_t[:, w0:w0 + gn, :],
                                           pseg[grp][:, 0:gn, :])
                            del pseg[grp]
                            if post_cb is not None:
                                post_cb(w0, gn)

            # ===== Phase C: layer-1 aggregation, passes 0..2 =====
            for q in range(3):
                aggregate(t1q_tab, HID, agg, q)

            # ===== Phase D machinery (runs in pass-3 callbacks) =====
            t2qbp = stD.enter_context(tc.tile_pool(name="t2qb", bufs=2))
            dstg = stD.enter_context(tc.tile_pool(name="dstg", bufs=2))
            psD = stD.enter_context(tc.tile_pool(name="psD", bufs=1,
                                                 space="PSUM"))
            psDt = stD.enter_context(tc.tile_pool(name="psDt", bufs=1,
                                                  space="PSUM"))
            # zero both rotating t2qb bufs once; cols C:128 stay zero
            for _ in range(2):
                z = t2qbp.tile([128, 25, 128], dt.bfloat16, tag="t2qb")
                nc.vector.memset(z[:], 0.0)

            dstate = {}

            def d_window(w):
                Q = 0 if w < 25 else (1 if w < 50 else (2 if w < 74 else 3))
                if dstate.get("Q") != Q:
                    dstate["Q"] = Q
                    dstate["t2qb"] = t2qbp.tile([128, 25, 128], dt.bfloat16,
                                                name="t2qb", tag="t2qb")
                i = w - QWSTART[Q]
                # s1: h1' = h1 + eps*tanh(aw@.. + dinv*agg)
                paw = psD.tile([128, HID], dt.float32, tag="paw")
                nc.tensor.matmul(paw[:], h1T_all[:, w, :], aw1T_sb[:],
                                 start=True, stop=True)
                pre = tmp_pool.tile([128, HID], dt.float32, tag="pre")
                nc.vector.scalar_tensor_tensor(
                    pre[:], agg[:, w, :], dinv_sb[:, w:w + 1], paw[:],
                    op0=Alu.mult, op1=Alu.add)
                if "bconv1" in bias_sb:
                    nc.vector.tensor_tensor(
                        pre[:], pre[:], bias_sb["bconv1"][:], op=Alu.add)
                th = tmp_pool.tile([128, HID], dt.float32, tag="th")
                nc.scalar.activation(th[:], pre[:], Act.Tanh)
                h1p = dstg.tile([128, HID], dt.bfloat16, tag="h1p")
                nc.vector.scalar_tensor_tensor(
                    h1p[:], th[:], EPS, h1[:, w, :],
                    op0=Alu.mult, op1=Alu.add)
                # s2: h2 = h1' @ lin2
                pt2 = psDt.tile([128, 128], dt.bfloat16, tag="ptD")
                nc.tensor.transpose(pt2[:], h1p[:], ident[:])
                h1pT = tmp_pool.tile([128, 128], dt.bfloat16, tag="h1pT")
                nc.scalar.copy(h1pT[:], pt2[:])
                ph2 = psD.tile([128, C], dt.float32, tag="pD32")
                nc.tensor.matmul(ph2[:], h1pT[:], lin2T_sb[:],
                                 start=True, stop=True)
                if "blin2" in bias_sb:
                    nc.vector.tensor_tensor(
                        h2[:, w, :], ph2[:], bias_sb["blin2"][:], op=Alu.add)
                else:
                    nc.scalar.copy(h2[:, w, :], ph2[:])
                # s3: h2T, T2 = dinv * (h2 @ phi2^T); agg2 self term
                p0 = (w % 2) * 64
                pt3 = psDt.tile([C, 128], dt.bfloat16, tag="pt3")
                nc.tensor.transpose(pt3[:], h2[:, w, :], ident[:])
                nc.scalar.copy(h2T_all[p0:p0 + C, w // 2, :], pt3[:])
                pT2 = psD.tile([128, C], dt.float32, tag="pD32")
                nc.tensor.matmul(pT2[:], h2T_all[p0:p0 + C, w // 2, :],
                                 phi2T_sb[p0:p0 + C, :],
                                 start=True, stop=True)
                nc.scalar.activation(dstate["t2qb"][:, i, 0:C], pT2[:],
                                     Act.Copy, scale=dinv_sb[:, w:w + 1])
                nc.scalar.activation(agg2[:, w, :], pT2[:],
                                     Act.Copy, scale=dinv_sb[:, w:w + 1])
                if w == QWEND[Q]:
                    nc.sync.dma_start(
                        t2q_in[Q][:].rearrange("(w p) f -> p w f", p=128),
                        dstate["t2qb"][:, 0:QW[Q], :])
                    nc.gpsimd.collective_compute(
                        "AllGather", Alu.bypass, replica_groups=rg,
                        ins=[t2q_in[Q][:].opt()], outs=[t2q_tab[Q][:].opt()])

            def d_cb(w0, gn):
                for i in range(gn):
                    d_window(w0 + i)

            # layer-1 pass 3 with phase-D callbacks
            aggregate(t1q_tab, HID, agg, 3, post_cb=d_cb)

            # ===== Phase G (runs in layer-2 pass-3 callbacks) =====
            def g_group(w0, gw):
                a1 = tmp_pool.tile([128, GRP, C], dt.float32, tag="a1g")
                nc.vector.tensor_tensor(
                    a1[:, 0:gw, :], agg2[:, w0:w0 + gw, :],
                    dinv_sb[:, w0:w0 + gw].unsqueeze(2)
                        .broadcast_to([128, gw, C]),
                    op=Alu.mult)
                pre = tmp_pool.tile([128, GRP, C], dt.float32, tag="preg")
                for wi in range(gw):
                    w = w0 + wi
                    p0 = (w % 2) * 64
                    pawt = psD.tile([128, C], dt.float32, tag="pD32")
                    nc.tensor.matmul(pawt[:],
                                     h2T_all[p0:p0 + C, w // 2, :],
                                     aw2T_sb[p0:p0 + C, :],
                                     start=True, stop=True)
                    nc.vector.tensor_tensor(
                        pre[:, wi, :], a1[:, wi, :], pawt[:], op=Alu.add)
                if "bconv2" in bias_sb:
                    nc.vector.tensor_tensor(
                        pre[:, 0:gw, :], pre[:, 0:gw, :],
                        bias_sb["bconv2"][:].unsqueeze(1)
                            .broadcast_to([128, gw, C]),
                        op=Alu.add)
                th = tmp_pool.tile([128, GRP, C], dt.float32, tag="thg")
                nc.scalar.activation(th[:, 0:gw, :], pre[:, 0:gw, :], Act.Tanh)
                h2p = tmp_pool.tile([128, GRP, C], dt.float32, tag="h2pg")
                nc.vector.scalar_tensor_tensor(
                    h2p[:, 0:gw, :], th[:, 0:gw, :], EPS,
                    h2[:, w0:w0 + gw, :], op0=Alu.mult, op1=Alu.add)
                negmax = tmp_pool.tile([128, GRP, 1], dt.float32, tag="nmg")
                nc.vector.tensor_reduce(negmax[:, 0:gw, :], h2p[:, 0:gw, :],
                                        axis=mybir.AxisListType.X,
                                        op=Alu.max, negate=True)
                sub = tmp_pool.tile([128, GRP, C], dt.float32, tag="subg")
                nc.vector.tensor_tensor(
                    sub[:, 0:gw, :], h2p[:, 0:gw, :],
                    negmax[:, 0:gw, :].broadcast_to([128, gw, C]),
                    op=Alu.add)
                e = tmp_pool.tile([128, GRP, C], dt.float32, tag="eg")
                nc.scalar.activation(e[:, 0:gw, :], sub[:, 0:gw, :], Act.Exp)
                ssum = tmp_pool.tile([128, GRP, 1], dt.float32, tag="ssg")
                nc.vector.tensor_reduce(ssum[:, 0:gw, :], e[:, 0:gw, :],
                                        axis=mybir.AxisListType.X,
                                        op=Alu.add)
                lse = tmp_pool.tile([128, GRP, 1], dt.float32, tag="lseg")
                nc.scalar.activation(lse[:, 0:gw, :], ssum[:, 0:gw, :], Act.Ln)
                fin = tmp_pool.tile([128, GRP, C], dt.float32, tag="fing")
                nc.vector.tensor_tensor(
                    fin[:, 0:gw, :], sub[:, 0:gw, :],
                    lse[:, 0:gw, :].broadcast_to([128, gw, C]),
                    op=Alu.subtract)
                nc.sync.dma_start(
                    out_p[w0 * 128:(w0 + gw) * 128, :]
                        .rearrange("(w p) c -> p w c", p=128),
                    fin[:, 0:gw, :])

            # ===== Phase F: layer-2 aggregation =====
            for q in range(4):
                aggregate(t2q_tab, C, agg2, q,
                          post_cb=g_group if q == 3 else None)

    nc.compile()
    return nc


def _call_op_ranges(ops, calls):
    """[o0, o1) op index range per gather call (ops sorted by tile)."""
    op_t = ops[0]
    ranges = []
    for t0, nt in calls:
        o0 = int(np.searchsorted(op_t, t0, side="left"))
        o1 = int(np.searchsorted(op_t, t0 + nt - 1, side="right"))
        ranges.append((o0, o1))
    return ranges


def kernel(**inputs):
    from concourse.bass_utils import run_bass_kernel_spmd

    inp = {k: np.asarray(v) for k, v in inputs.items()}
    in_maps, meta = _host_prep(**inp)

    key = ("graph", tuple(meta["LMAX"].tolist()),
           tuple(sorted(meta["use_bias"].items())))
    if key not in _CACHE:
        _CACHE[key] = _build_graph(meta)
    nc = _CACHE[key]

    import os
    trace = bool(int(os.environ.get("KERNEL_TRACE", "0")))
    res = run_bass_kernel_spmd(nc, in_maps, list(range(NCORES)), trace=trace,
                               tmpdir=os.environ.get("KERNEL_TRACE_DIR"))
    global LAST_EXEC_NS
    LAST_EXEC_NS = res.exec_time_ns

    out = np.concatenate([res.results[k]["out"][:SHARD] for k in range(NCORES)], 0)
    return out.astype(np.float32)


LAST_EXEC_NS = None
